# revision 1
# baseline (speedup 1.0000x reference)
"""DomainCalibratedLoss Trainium2 kernel.

loss = mean_n [ logsumexp_c(x[n,c] + log C[d_n,c]) - (x[n,t_n] + log C[d_n,t_n]) ]

v6 over v5:
  - B matmuls packed 2 tiles per instruction via a block-diagonal rhs:
    lhsT = [6,128] (two tiles' one-hots stacked, K=6), rhs = [6,400]
    (logc twice, block-diagonal), N=400 fits one PSUM bank.  4 matmuls
    per group instead of 8 -> halves PE LDW+dispatch overhead.
  - Optional hybrid add: the first PE_BLOCKS 400-wide blocks get z = B+x
    accumulated ON PE (identity matmul, start=False) and exp reads PSUM;
    the rest use the DVE add into bf16 SBUF.  Balances PE/DVE/ACT.
"""

import sys

sys.path.insert(0, "/opt/trn_rl_repo")

import numpy as np
import ml_dtypes

import concourse.bass as bass
import concourse.bacc as bacc
import concourse.tile as tile
from concourse import mybir
from concourse.bass_utils import run_bass_kernel_spmd
from concourse.masks import make_identity

P = 128          # partitions / points per tile
C = 200          # classes
D = 3            # domains
G = 8            # tiles per group (one x DMA)
NB = 4           # 400-wide blocks per group (2 tiles each)
BW = 2 * C       # block width = 400
BP = 512         # block stride in PSUM (bank)
GPP = 4          # groups per page (one oh DMA)
PT = G * GPP     # tiles per page = 32
N_CORES = 8

PE_BLOCKS = 2    # 400-wide blocks whose z=B+x is accumulated on PE

BF = mybir.dt.bfloat16
NPBF = ml_dtypes.bfloat16

_PROGRAM_CACHE = {}


def build_program(n_pages, reps=1):
    key = (n_pages, reps, PE_BLOCKS)
    if key in _PROGRAM_CACHE:
        return _PROGRAM_CACHE[key]

    T = n_pages * PT
    n_groups = T // G
    s_per = T * P

    nc = bacc.Bacc("TRN2", target_bir_lowering=False, debug=False,
                   num_devices=N_CORES)
    x_in = nc.dram_tensor("x", [n_groups * P, G * C], BF,
                          kind="ExternalInput").ap()
    # one-hot stacked pairs: per page [6, GPP*NB, P]
    oh_in = nc.dram_tensor("oh", [n_pages * 2 * D, GPP * NB * P], BF,
                           kind="ExternalInput").ap()
    sub_in = nc.dram_tensor("subt", [P, T], mybir.dt.float32,
                            kind="ExternalInput").ap()
    # block-diagonal logc: [6, 400]
    logc_in = nc.dram_tensor("logc2", [2 * D, BW], BF,
                             kind="ExternalInput").ap()
    r_out = nc.dram_tensor("r", [P, 1], mybir.dt.float32,
                           kind="ExternalOutput").ap()

    with tile.TileContext(nc) as tc:
        with (
            tc.tile_pool(name="singles", bufs=1) as singles,
            tc.tile_pool(name="xp", bufs=6) as xp,
            tc.tile_pool(name="ohp", bufs=3) as ohp,
            tc.tile_pool(name="ebp", bufs=3) as ebp,
            tc.tile_pool(name="zbp", bufs=3) as zbp,
            tc.tile_pool(name="psA", bufs=2, space="PSUM") as psA,
            tc.tile_pool(name="psB", bufs=2, space="PSUM") as psB,
        ):
            logc2 = singles.tile([2 * D, BW], BF)
            nc.sync.dma_start(out=logc2[:], in_=logc_in[:])
            sub_all = singles.tile([P, T], mybir.dt.float32)
            nc.sync.dma_start(out=sub_all[:], in_=sub_in[:])
            S_all = singles.tile([P, T], mybir.dt.float32)
            L_all = singles.tile([P, T], mybir.dt.float32)
            ident = None
            if PE_BLOCKS > 0:
                ident = singles.tile([P, P], BF)
                make_identity(nc, ident)

            def one_pass():
                pending = []

                def flush_reduce():
                    eb_p, lo = pending.pop(0)
                    nc.vector.tensor_reduce(
                        out=S_all[:, lo:lo + G],
                        in_=eb_p[:].rearrange("p (g c) -> p g c", c=C),
                        axis=mybir.AxisListType.X,
                        op=mybir.AluOpType.add)

                for pg in range(n_pages):
                    ohg = ohp.tile([2 * D, GPP * NB, P], BF, tag="oh")
                    nc.sync.dma_start(
                        out=ohg[:],
                        in_=oh_in[bass.ts(pg, 2 * D), :].rearrange(
                            "d (k p) -> d k p", p=P))
                    for g4 in range(GPP):
                        gi = pg * GPP + g4
                        xg = xp.tile([P, G * C], BF, tag="x")
                        nc.sync.dma_start(out=xg[:],
                                          in_=x_in[bass.ts(gi, P), :])
                        xv = xg[:].rearrange("p (k q) -> p k q", q=BW)
                        bzA = psA.tile([P, PE_BLOCKS, BP],
                                       mybir.dt.float32, tag="bzA")
                        bzB = psB.tile([P, NB - PE_BLOCKS, BP],
                                       mybir.dt.float32, tag="bzB")
                        # DVE-pair matmuls first so the add can start while
                        # PE still works on the PE-pair + identity matmuls.
                        for k in range(PE_BLOCKS, NB):
                            nc.tensor.matmul(bzB[:, k - PE_BLOCKS, 0:BW],
                                             lhsT=ohg[:, g4 * NB + k, :],
                                             rhs=logc2[:],
                                             start=True, stop=True)
                        eb = ebp.tile([P, G * C], BF, tag="e")
                        ev = eb[:].rearrange("p (k q) -> p k q", q=BW)
                        if PE_BLOCKS < NB:
                            zb = zbp.tile([P, (NB - PE_BLOCKS) * BW], BF,
                                          tag="z")
                            zv = zb[:].rearrange("p (k q) -> p k q", q=BW)
                            nc.vector.tensor_tensor(
                                out=zv[:],
                                in0=xv[:, PE_BLOCKS:NB, :],
                                in1=bzB[:, :, 0:BW],
                                op=mybir.AluOpType.add)
                        for k in range(PE_BLOCKS):
                            nc.tensor.matmul(bzA[:, k, 0:BW],
                                             lhsT=ohg[:, g4 * NB + k, :],
                                             rhs=logc2[:],
                                             start=True, stop=False)
                        for k in range(PE_BLOCKS):
                            nc.tensor.matmul(bzA[:, k, 0:BW],
                                             lhsT=ident[:],
                                             rhs=xv[:, k, :],
                                             start=False, stop=True)
                        # exp of the DVE half first: its input is ready
                        # before the PE-pair identity matmuls finish.
                        if PE_BLOCKS < NB:
                            nc.scalar.activation(
                                ev[:, PE_BLOCKS:NB, :], zv[:],
                                mybir.ActivationFunctionType.Exp)
                        if PE_BLOCKS > 0:
                            nc.scalar.activation(
                                ev[:, 0:PE_BLOCKS, :],
                                bzA[:, :, 0:BW],
                                mybir.ActivationFunctionType.Exp)
                        pending.append((eb, gi * G))
                        if len(pending) > 1:
                            flush_reduce()
                while pending:
                    flush_reduce()
                # epilogue: L = ln(S) - sub
                nc.scalar.activation(L_all[:], S_all[:],
                                     mybir.ActivationFunctionType.Ln)
                nc.vector.tensor_tensor(out=L_all[:], in0=L_all[:],
                                        in1=sub_all[:],
                                        op=mybir.AluOpType.subtract)

            if reps == 1:
                one_pass()
            else:
                with tc.For_i(0, reps):
                    one_pass()

            r = singles.tile([P, 1], mybir.dt.float32)
            nc.vector.tensor_reduce(out=r[:], in_=L_all[:],
                                    axis=mybir.AxisListType.X,
                                    op=mybir.AluOpType.add)
            nc.sync.dma_start(out=r_out[:], in_=r[:])

    nc.compile()
    _PROGRAM_CACHE[key] = nc
    return nc


def _host_prep(inputs, targets, domains, domain_counts, n_pages):
    """Build the per-core input maps (host-side sharding/marshalling)."""
    n = inputs.shape[0]
    T = n_pages * PT
    s_per = T * P
    n_groups = T // G
    n_pad = N_CORES * s_per

    logc = np.log(domain_counts.astype(np.float32)).astype(np.float32)
    tgt = targets.astype(np.int64).reshape(-1)
    dom = domains.astype(np.int64).reshape(-1)

    sub = np.empty(n_pad, dtype=np.float32)
    sub[:n] = inputs[np.arange(n), tgt] + logc[dom, tgt]
    sub[n:] = np.float32(np.log(float(C)))

    dom_pad = np.full(n_pad, -1, dtype=np.int64)
    dom_pad[:n] = dom

    # block-diagonal logc [6, 400]
    logc2 = np.zeros((2 * D, BW), dtype=np.float32)
    logc2[0:D, 0:C] = logc
    logc2[D:2 * D, C:2 * C] = logc

    in_maps = []
    for c in range(N_CORES):
        lo = c * s_per
        x_c = np.zeros((s_per, C), dtype=np.float32)
        n_real = max(0, min(s_per, n - lo))
        if n_real:
            x_c[:n_real] = inputs[lo:lo + n_real]
        x_m = np.ascontiguousarray(
            x_c.reshape(n_groups, G, P, C).transpose(0, 2, 1, 3)
        ).reshape(n_groups * P, G * C).astype(NPBF)
        dom_c = dom_pad[lo:lo + s_per]
        # oh[pg, r'*3+d, (g4*NB+k), p] = dom[tile g4*8+2k+r', p] == d
        dom_t = dom_c.reshape(n_pages, PT, P)       # [pg, tile, p]
        # tile index = (g4*NB + k)*2 + r' where within page; map tiles ->
        # [pg, pair, r', p] with pair = tile//2, r' = tile%2
        dom_pair = dom_t.reshape(n_pages, PT // 2, 2, P)
        oh = (dom_pair[:, None, :, :, :]
              == np.arange(D).reshape(1, D, 1, 1, 1))   # [pg, d, pair, r', p]
        # -> [pg, r'*D+d, pair, p]
        oh = oh.transpose(0, 3, 1, 2, 4).astype(NPBF)   # [pg, r', d, pair, p]
        oh = oh.reshape(n_pages, 2 * D, (PT // 2) * P)
        sub_c = np.ascontiguousarray(
            sub[lo:lo + s_per].reshape(T, P).T)
        in_maps.append({
            "x": x_m,
            "oh": np.ascontiguousarray(oh.reshape(n_pages * 2 * D,
                                                  (PT // 2) * P)),
            "subt": sub_c,
            "logc2": logc2.astype(NPBF),
        })
    return in_maps


def kernel(inputs, targets, domains, domain_counts):
    inputs = np.asarray(inputs, dtype=np.float32)
    targets_np = np.asarray(targets).reshape(-1)
    domains_np = np.asarray(domains).reshape(-1)
    counts = np.asarray(domain_counts, dtype=np.float32)

    n = inputs.shape[0]
    n_pages = -(-n // (N_CORES * PT * P))            # ceil -> 31 for N=1M

    nc = build_program(n_pages, reps=1)
    in_maps = _host_prep(inputs, targets_np, domains_np, counts, n_pages)
    res = run_bass_kernel_spmd(nc, in_maps, list(range(N_CORES)))

    total = 0.0
    for r in res.results:
        total += r["r"].astype(np.float64).sum()
    n_valid = int((targets_np != 255).sum())
    return np.float32(total / n_valid)



# revision 2
# speedup vs baseline: 1.4496x; 1.4496x over previous
"""DomainCalibratedLoss Trainium2 kernel.

loss = mean_n [ logsumexp_c(x[n,c] + log C[d_n,c]) - (x[n,t_n] + log C[d_n,t_n]) ]

v7 (transposed rewrite over v6):
  - Layout: classes on PARTITIONS (two chunks: 128 + 72), points along the
    free axis.  S[n] = sum_c C[d_n,c] * exp(x[c,n]) needs no logc gather and
    no big DVE class-reduce.
  - exp via Schraudolph fast-exp on DVE tensor_scalar (4x_2p mode, 0.26
    ns/col): E' = int16(round(A*x + B)), bit-reinterpreted as bf16.
  - class-sum via PE matmul: lhsT = E'[chunk, 128 points] (weights),
    rhs = counts^T[chunk, 3] -> PSUM [128 points, 3], accumulated over the
    two class chunks.  Domain selection = one-hot multiply + tiny reduce.
  - epilogue: L = ln(S_sel) - sub (sub = x[n,t]+logc[d,t] precomputed on
    host in f32), final per-partition reduce.
"""

import sys

sys.path.insert(0, "/opt/trn_rl_repo")

import numpy as np
import ml_dtypes

import concourse.bass as bass
import concourse.bacc as bacc
import concourse.tile as tile
from concourse import mybir
from concourse.bass_utils import run_bass_kernel_spmd

P = 128          # points per tile (partition dim of PSUM output)
C = 200          # classes
CA = 128         # class chunk A (partitions of xa)
CB = C - CA      # class chunk B = 72
D = 3            # domains
PT = 32          # tiles per "page" (kept for test.py's n_pages computation)
CT = 31          # tiles per chunk (one DMA / tensor_scalar span)
N_CORES = 8

FE_A = 128.0 / float(np.log(2.0))      # Schraudolph scale for bf16 bitcast
FE_B = 16248.633652670895              # tuned offset (zero mean ratio bias)

BF = mybir.dt.bfloat16
NPBF = ml_dtypes.bfloat16

_PROGRAM_CACHE = {}


def build_program(n_pages, reps=1):
    key = (n_pages, reps)
    if key in _PROGRAM_CACHE:
        return _PROGRAM_CACHE[key]

    T = n_pages * PT                  # tiles per core
    NP = T * P                        # points per core

    # chunk list: (tile_start, n_tiles)
    chunks = []
    t0 = 0
    while t0 < T:
        ct = min(CT, T - t0)
        chunks.append((t0, ct))
        t0 += ct

    nc = bacc.Bacc("TRN2", target_bir_lowering=False, debug=False,
                   num_devices=N_CORES)
    xa_in = nc.dram_tensor("xa", [CA, NP], BF, kind="ExternalInput").ap()
    xb_in = nc.dram_tensor("xb", [CB, NP], BF, kind="ExternalInput").ap()
    oh_in = nc.dram_tensor("oh", [P, D * T], BF, kind="ExternalInput").ap()
    sub_in = nc.dram_tensor("subt", [P, T], mybir.dt.float32,
                            kind="ExternalInput").ap()
    cwa_in = nc.dram_tensor("cwa", [CA, D], BF, kind="ExternalInput").ap()
    cwb_in = nc.dram_tensor("cwb", [CB, D], BF, kind="ExternalInput").ap()
    r_out = nc.dram_tensor("r", [P, 1], mybir.dt.float32,
                           kind="ExternalOutput").ap()

    with tile.TileContext(nc) as tc:
        with (
            tc.tile_pool(name="singles", bufs=1) as singles,
            tc.tile_pool(name="xap", bufs=3) as xap,
            tc.tile_pool(name="xbp", bufs=3) as xbp,
            tc.tile_pool(name="eap", bufs=3) as eap,
            tc.tile_pool(name="ebp", bufs=3) as ebp,
            tc.tile_pool(name="smp", bufs=3) as smp,
            tc.tile_pool(name="psp", bufs=3, space="PSUM") as psp,
        ):
            oh_all = singles.tile([P, D * T], BF)
            nc.sync.dma_start(out=oh_all[:], in_=oh_in[:])
            sub_all = singles.tile([P, T], mybir.dt.float32)
            nc.sync.dma_start(out=sub_all[:], in_=sub_in[:])
            cwa = singles.tile([CA, D], BF)
            nc.sync.dma_start(out=cwa[:], in_=cwa_in[:])
            cwb = singles.tile([CB, D], BF)
            nc.sync.dma_start(out=cwb[:], in_=cwb_in[:])
            S_all = singles.tile([P, T], mybir.dt.float32)
            L_all = singles.tile([P, T], mybir.dt.float32)

            def one_pass():
                for (t0, ct) in chunks:
                    F = ct * P
                    col0 = t0 * P
                    xa_t = xap.tile([CA, F], BF, tag="xa")
                    nc.sync.dma_start(out=xa_t[:],
                                      in_=xa_in[:, col0:col0 + F])
                    xb_t = xbp.tile([CB, F], BF, tag="xb")
                    nc.sync.dma_start(out=xb_t[:],
                                      in_=xb_in[:, col0:col0 + F])
                    ea = eap.tile([CA, F], mybir.dt.int16, tag="ea")
                    eb = ebp.tile([CB, F], mybir.dt.int16, tag="eb")
                    with nc.allow_low_precision(reason="fastexp bitcast"):
                        nc.vector.tensor_scalar(
                            out=ea[:], in0=xa_t[:], scalar1=FE_A,
                            scalar2=FE_B, op0=mybir.AluOpType.mult,
                            op1=mybir.AluOpType.add)
                        nc.vector.tensor_scalar(
                            out=eb[:], in0=xb_t[:], scalar1=FE_A,
                            scalar2=FE_B, op0=mybir.AluOpType.mult,
                            op1=mybir.AluOpType.add)
                    ps = psp.tile([P, 512], mybir.dt.float32, tag="ps")
                    for t in range(ct):
                        nc.tensor.matmul(
                            ps[:, 3 * t:3 * t + 3],
                            lhsT=ea[:, t * P:(t + 1) * P].bitcast(BF),
                            rhs=cwa[:], start=True, stop=False)
                        nc.tensor.matmul(
                            ps[:, 3 * t:3 * t + 3],
                            lhsT=eb[:, t * P:(t + 1) * P].bitcast(BF),
                            rhs=cwb[:], start=False, stop=True)
                    sm = smp.tile([P, D * ct], mybir.dt.float32, tag="sm")
                    nc.vector.tensor_tensor(
                        out=sm[:], in0=ps[:, 0:D * ct],
                        in1=oh_all[:, D * t0:D * (t0 + ct)],
                        op=mybir.AluOpType.mult)
                    nc.vector.tensor_reduce(
                        out=S_all[:, t0:t0 + ct],
                        in_=sm[:].rearrange("p (t d) -> p t d", d=D),
                        axis=mybir.AxisListType.X,
                        op=mybir.AluOpType.add)
                # epilogue: L = ln(S) - sub
                nc.scalar.activation(L_all[:], S_all[:],
                                     mybir.ActivationFunctionType.Ln)
                nc.vector.tensor_tensor(out=L_all[:], in0=L_all[:],
                                        in1=sub_all[:],
                                        op=mybir.AluOpType.subtract)

            if reps == 1:
                one_pass()
            else:
                with tc.For_i(0, reps):
                    one_pass()

            r = singles.tile([P, 1], mybir.dt.float32)
            nc.vector.tensor_reduce(out=r[:], in_=L_all[:],
                                    axis=mybir.AxisListType.X,
                                    op=mybir.AluOpType.add)
            nc.sync.dma_start(out=r_out[:], in_=r[:])

    nc.compile()
    _PROGRAM_CACHE[key] = nc
    return nc


def _fe0():
    """fastexp(0) exactly as the kernel computes it."""
    y = np.int16(np.rint(np.float32(FE_B)))
    return float(np.asarray(y, dtype=np.int16).view(NPBF))


def _host_prep(inputs, targets, domains, domain_counts, n_pages):
    """Build the per-core input maps (host-side sharding/marshalling)."""
    n = inputs.shape[0]
    T = n_pages * PT
    s_per = T * P
    n_pad = N_CORES * s_per

    counts_bf = domain_counts.astype(np.float32).astype(NPBF)
    logc = np.log(domain_counts.astype(np.float32)).astype(np.float32)
    tgt = targets.astype(np.int64).reshape(-1)
    dom = domains.astype(np.int64).reshape(-1)
    valid = tgt != 255

    # padding / invalid points: x = 0, domain 0, sub = ln(sum_c C~[0,c]*fe0)
    sub_pad = np.float32(np.log(np.sum(
        counts_bf[0].astype(np.float32) * np.float32(_fe0()),
        dtype=np.float32)))

    sub = np.full(n_pad, sub_pad, dtype=np.float32)
    tgt_v = np.where(valid, tgt, 0)
    sub_real = (inputs[np.arange(n), tgt_v].astype(np.float32)
                + logc[dom, tgt_v])
    sub[:n] = np.where(valid, sub_real, sub_pad)

    dom_pad = np.zeros(n_pad, dtype=np.int64)
    dom_pad[:n] = np.where(valid, dom, 0)

    x_bf = inputs.astype(np.float32).astype(NPBF)
    cw = np.ascontiguousarray(counts_bf.T)          # [C, D]

    in_maps = []
    for c in range(N_CORES):
        lo = c * s_per
        n_real = max(0, min(s_per, n - lo))
        x_c = np.zeros((s_per, C), dtype=NPBF)
        if n_real:
            x_c[:n_real] = x_bf[lo:lo + n_real]
            if not valid[lo:lo + n_real].all():
                x_c[:n_real][~valid[lo:lo + n_real]] = NPBF(0.0)
        x_t = np.ascontiguousarray(x_c.T)           # [C, s_per]
        dom_c = dom_pad[lo:lo + s_per].reshape(T, P)
        oh = (dom_c[:, :, None] == np.arange(D)[None, None, :])  # [T,P,D]
        oh = np.ascontiguousarray(
            oh.transpose(1, 0, 2).reshape(P, D * T)).astype(NPBF)
        sub_c = np.ascontiguousarray(sub[lo:lo + s_per].reshape(T, P).T)
        in_maps.append({
            "xa": np.ascontiguousarray(x_t[:CA]),
            "xb": np.ascontiguousarray(x_t[CA:]),
            "oh": oh,
            "subt": sub_c,
            "cwa": np.ascontiguousarray(cw[:CA]),
            "cwb": np.ascontiguousarray(cw[CA:]),
        })
    return in_maps


def kernel(inputs, targets, domains, domain_counts):
    inputs = np.asarray(inputs, dtype=np.float32)
    targets_np = np.asarray(targets).reshape(-1)
    domains_np = np.asarray(domains).reshape(-1)
    counts = np.asarray(domain_counts, dtype=np.float32)

    n = inputs.shape[0]
    n_pages = -(-n // (N_CORES * PT * P))            # ceil -> 31 for N=1M

    nc = build_program(n_pages, reps=1)
    in_maps = _host_prep(inputs, targets_np, domains_np, counts, n_pages)
    res = run_bass_kernel_spmd(nc, in_maps, list(range(N_CORES)))

    total = 0.0
    for r in res.results:
        total += r["r"].astype(np.float64).sum()
    n_valid = int((targets_np != 255).sum())
    return np.float32(total / n_valid)


# revision 3
# speedup vs baseline: 2.5806x; 1.7802x over previous
"""DomainCalibratedLoss Trainium2 kernel.

loss = mean_n [ logsumexp_c(x[n,c] + log C[d_n,c]) - (x[n,t_n] + log C[d_n,t_n]) ]

v8 (fp8 over v7):
  - Layout: classes on PARTITIONS (chunks of 128 + 72), points along free.
    S[n] = sum_c C[d_n,c] * exp(x[c,n]): no logc gather, no big DVE reduce.
  - x shipped as fp8 e4m3 -> halves HBM traffic AND makes PE the
    continuously-busy bottleneck so its clock stays at the 2.4 GHz p-state
    (idle gaps drop it to 1.2 GHz - measured 1.75x slowdown).
  - exp split across engines per chunk: DVE Schraudolph fast-exp
    (tensor_scalar fp8->int16, 2x_2p mode) or ACT real Exp (fp8->bf16,
    written into the int16 tile's bf16 bitcast view).
  - class-sum on PE: lhsT = E'[chunk, 128 points] as weights, rhs =
    counts^T[chunk, 3] -> PSUM [128, 3], accumulated over 2 class chunks.
  - epilogue: one-hot domain select, L = ln(S) - sub, per-partition reduce.
"""

import sys

sys.path.insert(0, "/opt/trn_rl_repo")

import numpy as np
import ml_dtypes

import concourse.bass as bass
import concourse.bacc as bacc
import concourse.tile as tile
from concourse import mybir
from concourse.bass_utils import run_bass_kernel_spmd

P = 128          # points per tile (partition dim of PSUM output)
C = 200          # classes
CA = 128         # class chunk A (partitions of xa)
CB = C - CA      # class chunk B = 72
D = 3            # domains
PT = 32          # tiles per "page" (kept for test.py's n_pages computation)
CT = 31          # tiles per chunk (one DMA / exp-instruction span)
N_CORES = 8

FE_A = 128.0 / float(np.log(2.0))      # Schraudolph scale for bf16 bitcast
FE_B = 16248.633652670895              # tuned offset (zero mean ratio bias)

BF = mybir.dt.bfloat16
FP8 = mybir.dt.float8e4
NPBF = ml_dtypes.bfloat16
NP8 = mybir.dt.np(FP8)

_PROGRAM_CACHE = {}

# ACT-exp chunks per 16 (rest use DVE fast-exp); balances ACT vs DVE busy.
_ACT_SET = frozenset({1, 3, 5, 7, 9, 11, 13})


def _is_act_chunk(k):
    return (k % 16) in _ACT_SET


def build_program(n_pages, reps=1):
    key = (n_pages, reps)
    if key in _PROGRAM_CACHE:
        return _PROGRAM_CACHE[key]

    T = n_pages * PT                  # tiles per core
    NP = T * P                        # points per core

    chunks = []
    t0 = 0
    while t0 < T:
        ct = min(CT, T - t0)
        chunks.append((t0, ct))
        t0 += ct

    nc = bacc.Bacc("TRN2", target_bir_lowering=False, debug=False,
                   num_devices=N_CORES)
    xa_in = nc.dram_tensor("xa", [CA, NP], FP8, kind="ExternalInput").ap()
    xb_in = nc.dram_tensor("xb", [CB, NP], FP8, kind="ExternalInput").ap()
    oh_in = nc.dram_tensor("oh", [P, D * T], BF, kind="ExternalInput").ap()
    sub_in = nc.dram_tensor("subt", [P, T], mybir.dt.float32,
                            kind="ExternalInput").ap()
    cwa_in = nc.dram_tensor("cwa", [CA, D], BF, kind="ExternalInput").ap()
    cwb_in = nc.dram_tensor("cwb", [CB, D], BF, kind="ExternalInput").ap()
    r_out = nc.dram_tensor("r", [P, 1], mybir.dt.float32,
                           kind="ExternalOutput").ap()

    with tile.TileContext(nc) as tc:
        with (
            tc.tile_pool(name="singles", bufs=1) as singles,
            tc.tile_pool(name="xap", bufs=4) as xap,
            tc.tile_pool(name="xbp", bufs=4) as xbp,
            tc.tile_pool(name="eap", bufs=4) as eap,
            tc.tile_pool(name="ebp", bufs=4) as ebp,
            tc.tile_pool(name="smp", bufs=3) as smp,
            tc.tile_pool(name="psp", bufs=3, space="PSUM") as psp,
        ):
            oh_all = singles.tile([P, D * T], BF)
            nc.sync.dma_start(out=oh_all[:], in_=oh_in[:])
            sub_all = singles.tile([P, T], mybir.dt.float32)
            nc.sync.dma_start(out=sub_all[:], in_=sub_in[:])
            cwa = singles.tile([CA, D], BF)
            nc.sync.dma_start(out=cwa[:], in_=cwa_in[:])
            cwb = singles.tile([CB, D], BF)
            nc.sync.dma_start(out=cwb[:], in_=cwb_in[:])
            S_all = singles.tile([P, T], mybir.dt.float32)
            L_all = singles.tile([P, T], mybir.dt.float32)

            def one_pass():
                for k, (t0, ct) in enumerate(chunks):
                    F = ct * P
                    col0 = t0 * P
                    xa_t = xap.tile([CA, F], FP8, tag="xa")
                    nc.sync.dma_start(out=xa_t[:],
                                      in_=xa_in[:, col0:col0 + F])
                    xb_t = xbp.tile([CB, F], FP8, tag="xb")
                    nc.sync.dma_start(out=xb_t[:],
                                      in_=xb_in[:, col0:col0 + F])
                    ea = eap.tile([CA, F], mybir.dt.int16, tag="ea")
                    eb = ebp.tile([CB, F], mybir.dt.int16, tag="eb")
                    if _is_act_chunk(k):
                        nc.scalar.activation(
                            ea[:].bitcast(BF), xa_t[:],
                            mybir.ActivationFunctionType.Exp)
                        nc.scalar.activation(
                            eb[:].bitcast(BF), xb_t[:],
                            mybir.ActivationFunctionType.Exp)
                    else:
                        with nc.allow_low_precision(reason="fastexp bitcast"):
                            nc.vector.tensor_scalar(
                                out=ea[:], in0=xa_t[:], scalar1=FE_A,
                                scalar2=FE_B, op0=mybir.AluOpType.mult,
                                op1=mybir.AluOpType.add)
                            nc.vector.tensor_scalar(
                                out=eb[:], in0=xb_t[:], scalar1=FE_A,
                                scalar2=FE_B, op0=mybir.AluOpType.mult,
                                op1=mybir.AluOpType.add)
                    ps = psp.tile([P, 512], mybir.dt.float32, tag="ps")
                    for t in range(ct):
                        nc.tensor.matmul(
                            ps[:, 3 * t:3 * t + 3],
                            lhsT=ea[:, t * P:(t + 1) * P].bitcast(BF),
                            rhs=cwa[:], start=True, stop=False)
                        nc.tensor.matmul(
                            ps[:, 3 * t:3 * t + 3],
                            lhsT=eb[:, t * P:(t + 1) * P].bitcast(BF),
                            rhs=cwb[:], start=False, stop=True)
                    sm = smp.tile([P, D * ct], mybir.dt.float32, tag="sm")
                    nc.vector.tensor_tensor(
                        out=sm[:], in0=ps[:, 0:D * ct],
                        in1=oh_all[:, D * t0:D * (t0 + ct)],
                        op=mybir.AluOpType.mult)
                    nc.vector.tensor_reduce(
                        out=S_all[:, t0:t0 + ct],
                        in_=sm[:].rearrange("p (t d) -> p t d", d=D),
                        axis=mybir.AxisListType.X,
                        op=mybir.AluOpType.add)
                # epilogue: L = ln(S) - sub
                nc.scalar.activation(L_all[:], S_all[:],
                                     mybir.ActivationFunctionType.Ln)
                nc.vector.tensor_tensor(out=L_all[:], in0=L_all[:],
                                        in1=sub_all[:],
                                        op=mybir.AluOpType.subtract)

            if reps == 1:
                one_pass()
            else:
                with tc.For_i(0, reps):
                    one_pass()

            r = singles.tile([P, 1], mybir.dt.float32)
            nc.vector.tensor_reduce(out=r[:], in_=L_all[:],
                                    axis=mybir.AxisListType.X,
                                    op=mybir.AluOpType.add)
            nc.sync.dma_start(out=r_out[:], in_=r[:])

    nc.compile()
    _PROGRAM_CACHE[key] = nc
    return nc


def _fe0():
    """fastexp(0) exactly as the kernel computes it."""
    y = np.int16(np.rint(np.float32(FE_B)))
    return float(np.asarray(y, dtype=np.int16).view(NPBF))


def _host_prep(inputs, targets, domains, domain_counts, n_pages):
    """Build the per-core input maps (host-side sharding/marshalling)."""
    n = inputs.shape[0]
    T = n_pages * PT
    s_per = T * P
    n_pad = N_CORES * s_per

    counts_bf = domain_counts.astype(np.float32).astype(NPBF)
    logc = np.log(domain_counts.astype(np.float32)).astype(np.float32)
    tgt = targets.astype(np.int64).reshape(-1)
    dom = domains.astype(np.int64).reshape(-1)
    valid = tgt != 255

    # padding / invalid points: x = 0, domain 0, sub = ln(sum_c C~[0,c]*e0)
    # where e0 = exp(0) as computed by that chunk's exp path.
    c0 = counts_bf[0].astype(np.float32)
    sub_pad_dve = np.float32(np.log(np.sum(c0 * np.float32(_fe0()),
                                           dtype=np.float32)))
    sub_pad_act = np.float32(np.log(np.sum(c0, dtype=np.float32)))

    # per-point chunk index within its core -> exp path
    idx_in_core = np.arange(n_pad, dtype=np.int64) % s_per
    chunk_of = idx_in_core // (CT * P)
    is_act = ((chunk_of % 16)[..., None] ==
              np.array(sorted(_ACT_SET))[None, :]).any(axis=1)
    sub_pad = np.where(is_act, sub_pad_act, sub_pad_dve).astype(np.float32)

    sub = sub_pad.copy()
    tgt_v = np.where(valid, tgt, 0)
    sub_real = (inputs[np.arange(n), tgt_v].astype(np.float32)
                + logc[dom, tgt_v])
    sub[:n] = np.where(valid, sub_real, sub_pad[:n])

    dom_pad = np.zeros(n_pad, dtype=np.int64)
    dom_pad[:n] = np.where(valid, dom, 0)

    x8 = inputs.astype(np.float32).astype(NP8)
    cw = np.ascontiguousarray(counts_bf.T)          # [C, D]

    in_maps = []
    for c in range(N_CORES):
        lo = c * s_per
        n_real = max(0, min(s_per, n - lo))
        x_c = np.zeros((s_per, C), dtype=NP8)
        if n_real:
            x_c[:n_real] = x8[lo:lo + n_real]
            if not valid[lo:lo + n_real].all():
                x_c[:n_real][~valid[lo:lo + n_real]] = NP8(0.0)
        x_t = np.ascontiguousarray(x_c.T)           # [C, s_per]
        dom_c = dom_pad[lo:lo + s_per].reshape(T, P)
        oh = (dom_c[:, :, None] == np.arange(D)[None, None, :])  # [T,P,D]
        oh = np.ascontiguousarray(
            oh.transpose(1, 0, 2).reshape(P, D * T)).astype(NPBF)
        sub_c = np.ascontiguousarray(sub[lo:lo + s_per].reshape(T, P).T)
        in_maps.append({
            "xa": np.ascontiguousarray(x_t[:CA]),
            "xb": np.ascontiguousarray(x_t[CA:]),
            "oh": oh,
            "subt": sub_c,
            "cwa": np.ascontiguousarray(cw[:CA]),
            "cwb": np.ascontiguousarray(cw[CA:]),
        })
    return in_maps


def kernel(inputs, targets, domains, domain_counts):
    inputs = np.asarray(inputs, dtype=np.float32)
    targets_np = np.asarray(targets).reshape(-1)
    domains_np = np.asarray(domains).reshape(-1)
    counts = np.asarray(domain_counts, dtype=np.float32)

    n = inputs.shape[0]
    n_pages = -(-n // (N_CORES * PT * P))            # ceil -> 31 for N=1M

    nc = build_program(n_pages, reps=1)
    in_maps = _host_prep(inputs, targets_np, domains_np, counts, n_pages)
    res = run_bass_kernel_spmd(nc, in_maps, list(range(N_CORES)))

    total = 0.0
    for r in res.results:
        total += r["r"].astype(np.float64).sum()
    n_valid = int((targets_np != 255).sum())
    return np.float32(total / n_valid)
